# revision 16
# baseline (speedup 1.0000x reference)
"""FECAM layer Trainium2 kernel.

Reference computation (per batch element b, X = x[b] in R^{512x512}, layout [l, c]):
    xp   = X^T                                  # [c, l]
    freq = xp @ D^T                             # DCT-II along l      [c, k]
    sd   = LN(freq) * gamma + beta              # LayerNorm over k
    h    = relu(sd @ W1^T)                      # [c, 2C]
    fw   = sigmoid(h @ W2^T)                    # [c, k]
    fw   = LN(fw) * gamma + beta
    out  = (xp * fw)^T = X .* fw^T              # [l, c]  (natural layout)

Device strategy (data parallel, 16 batch elements per core x 8 cores):
  - freq[c,k] = matmul(lhsT=x_b tiles [l,c], rhs=D^T tiles [l,k]) -> psum
  - LN1 stats via bn_stats/bn_aggr; rstd = Exp(-0.5*Ln(var+eps)) on ACT
    (keeps every activation in ONE table set - no ACT_TABLE_LOAD thrash);
    z = Identity(freq*rstd - mu*rstd) eviction on ACT
  - LN1 gamma/beta folded into fc1 on host: W1g[h,k]=w1[h,k]*gamma[k],
    b1[h]=sum_k beta[k]*w1[h,k]
  - z transposed 128x128 via PE (f32r, 1.5 cyc/row) into zT [k,c]
  - fc1: hT = relu(W1g @ zT + b1) in [h,c];  fc2: y = hT^T @ W2^T -> [c,k]
  - sigmoid = reciprocal_approx_fast(1 + Exp(-y)): Exp on ACT, +1 and recip on DVE
  - LN2 stats likewise; z2 = Identity eviction; transpose via PE;
    final affine (gamma/beta per-partition) on ACT; multiply by x on DVE
  - emission is software-pipelined with a 2-batch skew so the PE queue always
    has independent matmul work while a batch's LN/sigmoid chains complete:
      cycle b emits: DCT(b) | T2+final(b-2) | T(b-1) fc1(b-1) fc2(b-1)
All matmuls float32r: fp32 operands streamed at 1 cycle/row at free dim 512;
hardware rounds operands tf32-style -> rel err ~4e-4 vs fp64 reference.
"""

import sys

if "/opt/trn_rl_repo" not in sys.path:
    sys.path.insert(0, "/opt/trn_rl_repo")

import numpy as np

P = 128
C = 512          # channels == seq len == dct size
H = 1024         # hidden
CT = C // P      # 4 c-tiles
KT = C // P      # 4 k-tiles
HT = H // P      # 8 h-tiles
EPS = 1e-6
N_CORES = 8
B_FULL = 128

_NC_CACHE: dict = {}

MM_MODE = "f32r"


def _build(nb: int):
    import concourse.bass as bass
    from concourse import bacc
    import concourse.mybir as mybir
    from concourse.tile import TileContext

    f32 = mybir.dt.float32
    f32r = mybir.dt.float32r
    Relu = mybir.ActivationFunctionType.Relu
    Ln = mybir.ActivationFunctionType.Ln
    Exp = mybir.ActivationFunctionType.Exp
    Ident = mybir.ActivationFunctionType.Identity
    mult = mybir.AluOpType.mult
    add = mybir.AluOpType.add

    mdt = f32r if MM_MODE == "f32r" else f32

    nc = bacc.Bacc()
    x_d = nc.declare_dram_parameter("x", [nb, C, C], mdt, isOutput=False)
    dt_d = nc.declare_dram_parameter("dt", [C, C], mdt, isOutput=False)
    w1t_d = nc.declare_dram_parameter("w1t", [C, H], mdt, isOutput=False)
    b1_d = nc.declare_dram_parameter("b1", [H], f32, isOutput=False)
    w2t_d = nc.declare_dram_parameter("w2t", [H, C], mdt, isOutput=False)
    gb_d = nc.declare_dram_parameter("gb", [C, 2], f32, isOutput=False)
    id_d = nc.declare_dram_parameter("iden", [P, P], mdt, isOutput=False)
    out_d = nc.declare_dram_parameter("out", [nb, C, C], f32, isOutput=True)

    with TileContext(nc) as tc, \
            tc.tile_pool(name="consts", bufs=1) as consts, \
            tc.tile_pool(name="xin", bufs=4) as xin, \
            tc.tile_pool(name="work", bufs=2) as work, \
            tc.tile_pool(name="small", bufs=8) as small, \
            tc.tile_pool(name="res", bufs=4) as resp, \
            tc.tile_pool(name="ps_mm", bufs=2, space="PSUM") as ps_mm, \
            tc.tile_pool(name="ps_t", bufs=2, space="PSUM") as ps_t, \
            tc.tile_pool(name="ps_h", bufs=2, space="PSUM") as ps_h, \
            tc.tile_pool(name="ps_w", bufs=2, space="PSUM") as ps_w:

        # one ACT table set covering Ln/Exp/Identity/Copy/Relu: pre-seed it so
        # bacc's availability pass never inserts another load
        from concourse.hw_specs import get_activation_tables
        set_names = list(get_activation_tables(nc.m.arch))
        nc.scalar.add_instruction(mybir.InstLoadActFuncSet(
            name=nc.get_next_instruction_name(),
            act_func_set_id=set_names.index("natural_log_exp_and_others"),
            ins=[], outs=[]))

        dt_sb = consts.tile([P, KT, C], mdt)
        nc.sync.dma_start(out=dt_sb, in_=dt_d.rearrange("(t p) k -> p t k", p=P))
        w1t_sb = consts.tile([P, KT, H], mdt)
        nc.sync.dma_start(out=w1t_sb, in_=w1t_d.rearrange("(t p) h -> p t h", p=P))
        w2t_sb = consts.tile([P, HT, C], mdt)
        nc.sync.dma_start(out=w2t_sb, in_=w2t_d.rearrange("(t p) k -> p t k", p=P))
        b1_sb = consts.tile([P, HT], f32)
        nc.sync.dma_start(out=b1_sb, in_=b1_d.rearrange("(t p) -> p t", p=P))
        gb_sb = consts.tile([P, KT, 2], f32)
        nc.sync.dma_start(out=gb_sb, in_=gb_d.rearrange("(t p) g -> p t g", p=P))
        id_sb = consts.tile([P, P], mdt)
        nc.sync.dma_start(out=id_sb, in_=id_d[:])
        eps_sb = consts.tile([P, 1], f32)
        nc.vector.memset(eps_sb, EPS)

        st: dict = {}   # per-batch live tiles

        def ln_rstd_nmr(mv):
            """(rstd, -mu*rstd) from bn_aggr output, Ln/Exp on ACT."""
            lv = small.tile([P, 1], f32, tag="lv")
            nc.scalar.activation(out=lv, in_=mv[:, 1:2], func=Ln,
                                 bias=eps_sb, scale=1.0)
            rstd = small.tile([P, 1], f32, tag="rstd")
            nc.scalar.activation(out=rstd, in_=lv, func=Exp,
                                 bias=0.0, scale=-0.5)
            nmr = small.tile([P, 1], f32, tag="nmr")
            nc.vector.tensor_scalar(out=nmr, in0=mv[:, 0:1],
                                    scalar1=rstd, scalar2=-1.0,
                                    op0=mult, op1=mult)
            return rstd, nmr

        def emit_load(b):
            xb = xin.tile([P, KT, C], mdt, tag="xb")
            nc.sync.dma_start(out=xb, in_=x_d[b].rearrange("(t p) c -> p t c", p=P))
            st[b] = {"xb": xb}

        def emit_dct_ln1_group(b, mc):
            if mc == 0:
                z_new = work.tile([P, CT, C], mdt, tag="z")
                st[b]["z"] = z_new
            xb = st[b]["xb"]
            z = st[b]["z"]
            pf = ps_mm.tile([P, C], f32, tag="pf")
            for lt in range(KT):
                nc.tensor.matmul(
                    pf,
                    lhsT=xb[:, lt, mc * P:(mc + 1) * P],
                    rhs=dt_sb[:, lt, :],
                    start=(lt == 0),
                    stop=(lt == KT - 1),
                )
            stats = small.tile([P, 6], f32, tag="stats")
            nc.vector.bn_stats(out=stats, in_=pf)
            mv = small.tile([P, 2], f32, tag="mv")
            nc.vector.bn_aggr(out=mv, in_=stats)
            rstd, nmr = ln_rstd_nmr(mv)
            nc.scalar.activation(out=z[:, mc, :], in_=pf, func=Ident,
                                 bias=nmr, scale=rstd)

        def emit_t1_group(b, kt):
            if kt == 0:
                zT_new = work.tile([P, KT, C], mdt, tag="zT")
                st[b]["zT"] = zT_new
            z = st[b]["z"]
            zT = st[b]["zT"]
            pt = ps_t.tile([P, C], mdt, tag="pt")
            for mc in range(CT):
                nc.tensor.transpose(pt[:, mc * P:(mc + 1) * P],
                                    z[:, mc, kt * P:(kt + 1) * P], id_sb)
            nc.vector.tensor_copy(zT[:, kt, :], pt)
            if kt == KT - 1:
                del st[b]["z"]

        def emit_fc1_group(b, mh):
            if mh == 0:
                hT_new = work.tile([P, HT, C], mdt, tag="hT")
                st[b]["hT"] = hT_new
            zT = st[b]["zT"]
            hT = st[b]["hT"]
            ph = ps_h.tile([P, C], f32, tag="ph")
            for kt in range(KT):
                nc.tensor.matmul(
                    ph,
                    lhsT=w1t_sb[:, kt, mh * P:(mh + 1) * P],
                    rhs=zT[:, kt, :],
                    start=(kt == 0),
                    stop=(kt == KT - 1),
                )
            nc.scalar.activation(out=hT[:, mh, :], in_=ph, func=Relu,
                                 bias=b1_sb[:, mh:mh + 1], scale=1.0)
            if mh == HT - 1:
                del st[b]["zT"]

        def emit_fc2_ln2(b):
            hT = st[b].pop("hT")
            z2 = work.tile([P, CT, C], mdt, tag="z2")
            for mc in range(CT):
                pw = ps_w.tile([P, C], f32, tag="pw")
                for ht in range(HT):
                    nc.tensor.matmul(
                        pw,
                        lhsT=hT[:, ht, mc * P:(mc + 1) * P],
                        rhs=w2t_sb[:, ht, :],
                        start=(ht == 0),
                        stop=(ht == HT - 1),
                    )
                et = work.tile([P, C], f32, tag="et")
                nc.scalar.activation(out=et, in_=pw, func=Exp,
                                     bias=0.0, scale=-1.0)
                nc.vector.tensor_scalar_add(out=et, in0=et, scalar1=1.0)
                fwp = work.tile([P, C], f32, tag="fwp")
                nc.vector.reciprocal_approx_fast(out=fwp, in_=et)
                stats2 = small.tile([P, 6], f32, tag="stats")
                nc.vector.bn_stats(out=stats2, in_=fwp)
                mv2 = small.tile([P, 2], f32, tag="mv")
                nc.vector.bn_aggr(out=mv2, in_=stats2)
                rstd2, nmr2 = ln_rstd_nmr(mv2)
                nc.scalar.activation(out=z2[:, mc, :], in_=fwp, func=Ident,
                                     bias=nmr2, scale=rstd2)
            st[b]["z2"] = z2

        def emit_t2_final_group(b, kt):
            z2 = st[b]["z2"]
            xb = st[b]["xb"]
            pt2 = ps_t.tile([P, C], mdt, tag="pt")
            for mc in range(CT):
                nc.tensor.transpose(pt2[:, mc * P:(mc + 1) * P],
                                    z2[:, mc, kt * P:(kt + 1) * P], id_sb)
            res = resp.tile([P, C], f32, tag="res")
            nc.vector.tensor_scalar(out=res, in0=pt2,
                                    scalar1=gb_sb[:, kt, 0:1],
                                    scalar2=gb_sb[:, kt, 1:2],
                                    op0=mult, op1=add)
            nc.vector.tensor_mul(out=res, in0=res, in1=xb[:, kt, :])
            nc.sync.dma_start(out=out_d[b, kt * P:(kt + 1) * P, :], in_=res)
            if kt == KT - 1:
                del st[b]

        # software pipeline, 2-batch skew, with transpose groups woven
        # between independent matmul groups so their psum evictions are
        # hidden behind PE work instead of stalling the pt slots:
        #   cycle b: DCT(b) x T1(b-1) | fc1(b-1) x T2(b-2) | fc2(b-1)
        for b in range(nb + 2):
            if b < nb:
                emit_load(b)
            for g in range(max(CT, KT)):
                if b < nb:
                    emit_dct_ln1_group(b, g)
                if 1 <= b <= nb:
                    emit_t1_group(b - 1, g)
            for mh in range(HT):
                if 1 <= b <= nb:
                    emit_fc1_group(b - 1, mh)
                if b >= 2 and mh % 2 == 1:
                    emit_t2_final_group(b - 2, mh // 2)
            if 1 <= b <= nb:
                emit_fc2_ln2(b - 1)

    # Bacc's compile passes (register alloc, wait splitting for fp32 matmuls)
    # run in finalize(); the pjrt exec path requires a finalized module.
    nc.finalize()
    return nc


def get_nc(nb: int):
    key = (nb, MM_MODE)
    if key not in _NC_CACHE:
        _NC_CACHE[key] = _build(nb)
    return _NC_CACHE[key]


def make_host_inputs(x, gamma, beta, w1, w2):
    """Host-side precompute: DCT matrix + folded weights."""
    x = np.ascontiguousarray(np.asarray(x, dtype=np.float32))
    gamma = np.asarray(gamma, dtype=np.float32)
    beta = np.asarray(beta, dtype=np.float32)
    w1 = np.asarray(w1, dtype=np.float32)
    w2 = np.asarray(w2, dtype=np.float32)

    k = np.arange(C)[:, None].astype(np.float64)
    m = np.arange(C)[None, :].astype(np.float64)
    D = 2.0 * np.cos(np.pi * k * (2.0 * m + 1.0) / (2.0 * C))  # [k, l]
    dt = np.ascontiguousarray(D.T.astype(np.float32))           # [l, k]

    w1t = np.ascontiguousarray((w1 * gamma[None, :]).T)         # [k, h]
    b1 = (w1 @ beta).astype(np.float32)                         # [h]
    w2t = np.ascontiguousarray(w2.T)                            # [h, k]
    gb = np.ascontiguousarray(np.stack([gamma, beta], axis=1))  # [k, 2]
    iden = np.eye(P, dtype=np.float32)
    return x, dict(dt=dt, w1t=w1t, b1=b1, w2t=w2t, gb=gb, iden=iden)


def make_in_maps(x, const):
    nb = B_FULL // N_CORES
    return [dict(x=x[i * nb:(i + 1) * nb], **const) for i in range(N_CORES)]


def kernel(x, gamma, beta, w1, w2):
    from concourse.bass_utils import run_bass_kernel_spmd

    x, const = make_host_inputs(x, gamma, beta, w1, w2)
    nc = get_nc(B_FULL // N_CORES)
    in_maps = make_in_maps(x, const)
    r = run_bass_kernel_spmd(nc, in_maps, list(range(N_CORES)))
    return np.concatenate([r.results[i]["out"] for i in range(N_CORES)], axis=0)


# revision 19
# speedup vs baseline: 1.0270x; 1.0270x over previous
"""FECAM layer Trainium2 kernel.

Reference computation (per batch element b, X = x[b] in R^{512x512}, layout [l, c]):
    xp   = X^T                                  # [c, l]
    freq = xp @ D^T                             # DCT-II along l      [c, k]
    sd   = LN(freq) * gamma + beta              # LayerNorm over k
    h    = relu(sd @ W1^T)                      # [c, 2C]
    fw   = sigmoid(h @ W2^T)                    # [c, k]
    fw   = LN(fw) * gamma + beta
    out  = (xp * fw)^T = X .* fw^T              # [l, c]  (natural layout)

Device strategy (data parallel, 16 batch elements per core x 8 cores):
  - freq[c,k] = matmul(lhsT=x_b tiles [l,c], rhs=D^T tiles [l,k]) -> psum
  - LN1 stats via bn_stats/bn_aggr; rstd = Exp(-0.5*Ln(var+eps)) on ACT
    (keeps every activation in ONE table set - no ACT_TABLE_LOAD thrash);
    z = Identity(freq*rstd - mu*rstd) eviction on ACT
  - LN1 gamma/beta folded into fc1 on host: W1g[h,k]=w1[h,k]*gamma[k],
    b1[h]=sum_k beta[k]*w1[h,k]
  - z transposed 128x128 via PE (f32r, 1.5 cyc/row) into zT [k,c]
  - fc1: hT = relu(W1g @ zT + b1) in [h,c];  fc2: y = hT^T @ W2^T -> [c,k]
  - sigmoid = reciprocal_approx_fast(1 + Exp(-y)): Exp on ACT, +1 and recip on DVE
  - LN2 stats likewise; z2 = Identity eviction; transpose via PE;
    final affine (gamma/beta per-partition) on ACT; multiply by x on DVE
  - emission is software-pipelined with a 2-batch skew so the PE queue always
    has independent matmul work while a batch's LN/sigmoid chains complete:
      cycle b emits: DCT(b) | T2+final(b-2) | T(b-1) fc1(b-1) fc2(b-1)
All matmuls float32r: fp32 operands streamed at 1 cycle/row at free dim 512;
hardware rounds operands tf32-style -> rel err ~4e-4 vs fp64 reference.
"""

import sys

if "/opt/trn_rl_repo" not in sys.path:
    sys.path.insert(0, "/opt/trn_rl_repo")

import numpy as np

P = 128
C = 512          # channels == seq len == dct size
H = 1024         # hidden
CT = C // P      # 4 c-tiles
KT = C // P      # 4 k-tiles
HT = H // P      # 8 h-tiles
EPS = 1e-6
N_CORES = 8
B_FULL = 128

_NC_CACHE: dict = {}

MM_MODE = "f32r"


def _build(nb: int):
    import concourse.bass as bass
    from concourse import bacc
    import concourse.mybir as mybir
    from concourse.tile import TileContext

    f32 = mybir.dt.float32
    f32r = mybir.dt.float32r
    Relu = mybir.ActivationFunctionType.Relu
    Ln = mybir.ActivationFunctionType.Ln
    Exp = mybir.ActivationFunctionType.Exp
    Ident = mybir.ActivationFunctionType.Identity
    mult = mybir.AluOpType.mult
    add = mybir.AluOpType.add

    mdt = f32r if MM_MODE == "f32r" else f32

    nc = bacc.Bacc()
    x_d = nc.declare_dram_parameter("x", [nb, C, C], mdt, isOutput=False)
    dt_d = nc.declare_dram_parameter("dt", [C, C], mdt, isOutput=False)
    w1t_d = nc.declare_dram_parameter("w1t", [C, H], mdt, isOutput=False)
    b1_d = nc.declare_dram_parameter("b1", [H], f32, isOutput=False)
    w2t_d = nc.declare_dram_parameter("w2t", [H, C], mdt, isOutput=False)
    gb_d = nc.declare_dram_parameter("gb", [C, 2], f32, isOutput=False)
    id_d = nc.declare_dram_parameter("iden", [P, P], mdt, isOutput=False)
    out_d = nc.declare_dram_parameter("out", [nb, C, C], f32, isOutput=True)

    with TileContext(nc) as tc, \
            tc.tile_pool(name="consts", bufs=1) as consts, \
            tc.tile_pool(name="xin", bufs=4) as xin, \
            tc.tile_pool(name="work", bufs=2) as work, \
            tc.tile_pool(name="small", bufs=8) as small, \
            tc.tile_pool(name="res", bufs=4) as resp, \
            tc.tile_pool(name="sig", bufs=4) as sig, \
            tc.tile_pool(name="ps_mm", bufs=2, space="PSUM") as ps_mm, \
            tc.tile_pool(name="ps_t", bufs=2, space="PSUM") as ps_t, \
            tc.tile_pool(name="ps_h", bufs=2, space="PSUM") as ps_h, \
            tc.tile_pool(name="ps_w", bufs=2, space="PSUM") as ps_w:

        # one ACT table set covering Ln/Exp/Identity/Copy/Relu: pre-seed it so
        # bacc's availability pass never inserts another load
        from concourse.hw_specs import get_activation_tables
        set_names = list(get_activation_tables(nc.m.arch))
        nc.scalar.add_instruction(mybir.InstLoadActFuncSet(
            name=nc.get_next_instruction_name(),
            act_func_set_id=set_names.index("natural_log_exp_and_others"),
            ins=[], outs=[]))

        dt_sb = consts.tile([P, KT, C], mdt)
        nc.sync.dma_start(out=dt_sb, in_=dt_d.rearrange("(t p) k -> p t k", p=P))
        w1t_sb = consts.tile([P, KT, H], mdt)
        nc.sync.dma_start(out=w1t_sb, in_=w1t_d.rearrange("(t p) h -> p t h", p=P))
        w2t_sb = consts.tile([P, HT, C], mdt)
        nc.sync.dma_start(out=w2t_sb, in_=w2t_d.rearrange("(t p) k -> p t k", p=P))
        b1_sb = consts.tile([P, HT], f32)
        nc.sync.dma_start(out=b1_sb, in_=b1_d.rearrange("(t p) -> p t", p=P))
        gb_sb = consts.tile([P, KT, 2], f32)
        nc.sync.dma_start(out=gb_sb, in_=gb_d.rearrange("(t p) g -> p t g", p=P))
        id_sb = consts.tile([P, P], mdt)
        nc.sync.dma_start(out=id_sb, in_=id_d[:])
        eps_sb = consts.tile([P, 1], f32)
        nc.vector.memset(eps_sb, EPS)

        st: dict = {}   # per-batch live tiles

        def ln_rstd_nmr(mv):
            """(rstd, -mu*rstd) from bn_aggr output, Ln/Exp on ACT."""
            lv = small.tile([P, 1], f32, tag="lv")
            nc.scalar.activation(out=lv, in_=mv[:, 1:2], func=Ln,
                                 bias=eps_sb, scale=1.0)
            rstd = small.tile([P, 1], f32, tag="rstd")
            nc.scalar.activation(out=rstd, in_=lv, func=Exp,
                                 bias=0.0, scale=-0.5)
            nmr = small.tile([P, 1], f32, tag="nmr")
            nc.vector.tensor_scalar(out=nmr, in0=mv[:, 0:1],
                                    scalar1=rstd, scalar2=-1.0,
                                    op0=mult, op1=mult)
            return rstd, nmr

        def emit_load(b):
            xb = xin.tile([P, KT, C], mdt, tag="xb")
            nc.sync.dma_start(out=xb, in_=x_d[b].rearrange("(t p) c -> p t c", p=P))
            st[b] = {"xb": xb}

        def emit_dct_ln1_group(b, mc):
            if mc == 0:
                z_new = work.tile([P, CT, C], mdt, tag="z")
                st[b]["z"] = z_new
            xb = st[b]["xb"]
            z = st[b]["z"]
            pf = ps_mm.tile([P, C], f32, tag="pf")
            for lt in range(KT):
                nc.tensor.matmul(
                    pf,
                    lhsT=xb[:, lt, mc * P:(mc + 1) * P],
                    rhs=dt_sb[:, lt, :],
                    start=(lt == 0),
                    stop=(lt == KT - 1),
                )
            stats = small.tile([P, 6], f32, tag="stats")
            nc.vector.bn_stats(out=stats, in_=pf)
            mv = small.tile([P, 2], f32, tag="mv")
            nc.vector.bn_aggr(out=mv, in_=stats)
            rstd, nmr = ln_rstd_nmr(mv)
            nc.scalar.activation(out=z[:, mc, :], in_=pf, func=Ident,
                                 bias=nmr, scale=rstd)

        def emit_t1_group(b, kt):
            if kt == 0:
                zT_new = work.tile([P, KT, C], mdt, tag="zT")
                st[b]["zT"] = zT_new
            z = st[b]["z"]
            zT = st[b]["zT"]
            pt = ps_t.tile([P, C], mdt, tag="pt")
            for mc in range(CT):
                nc.tensor.transpose(pt[:, mc * P:(mc + 1) * P],
                                    z[:, mc, kt * P:(kt + 1) * P], id_sb)
            nc.scalar.copy(out=zT[:, kt, :], in_=pt)
            if kt == KT - 1:
                del st[b]["z"]

        def emit_fc1_group(b, mh):
            if mh == 0:
                hT_new = work.tile([P, HT, C], mdt, tag="hT")
                st[b]["hT"] = hT_new
            zT = st[b]["zT"]
            hT = st[b]["hT"]
            ph = ps_h.tile([P, C], f32, tag="ph")
            for kt in range(KT):
                nc.tensor.matmul(
                    ph,
                    lhsT=w1t_sb[:, kt, mh * P:(mh + 1) * P],
                    rhs=zT[:, kt, :],
                    start=(kt == 0),
                    stop=(kt == KT - 1),
                )
            nc.scalar.activation(out=hT[:, mh, :], in_=ph, func=Relu,
                                 bias=b1_sb[:, mh:mh + 1], scale=1.0)
            if mh == HT - 1:
                del st[b]["zT"]

        def emit_fc2_ln2(b):
            hT = st[b].pop("hT")
            z2 = work.tile([P, CT, C], mdt, tag="z2")
            mvs = small.tile([P, CT, 2], f32, tag="mvs")
            fwps = []
            for mc in range(CT):
                pw = ps_w.tile([P, C], f32, tag="pw")
                for ht in range(HT):
                    nc.tensor.matmul(
                        pw,
                        lhsT=hT[:, ht, mc * P:(mc + 1) * P],
                        rhs=w2t_sb[:, ht, :],
                        start=(ht == 0),
                        stop=(ht == HT - 1),
                    )
                et = work.tile([P, C], f32, tag="et")
                nc.scalar.activation(out=et, in_=pw, func=Exp,
                                     bias=0.0, scale=-1.0)
                nc.vector.tensor_scalar_add(out=et, in0=et, scalar1=1.0)
                fwp = sig.tile([P, C], f32, tag="fwp")
                nc.vector.reciprocal_approx_fast(out=fwp, in_=et)
                stats2 = small.tile([P, 6], f32, tag="stats")
                nc.vector.bn_stats(out=stats2, in_=fwp)
                nc.vector.bn_aggr(out=mvs[:, mc, :], in_=stats2)
                fwps.append(fwp)
            # batched rstd/-mu*rstd for all 4 chunks: one Ln + one Exp + one TS
            lv4 = small.tile([P, CT, 1], f32, tag="lv4")
            nc.scalar.activation(out=lv4, in_=mvs[:, :, 1:2], func=Ln,
                                 bias=eps_sb, scale=1.0)
            rstd4 = small.tile([P, CT, 1], f32, tag="rstd4")
            nc.scalar.activation(out=rstd4, in_=lv4, func=Exp,
                                 bias=0.0, scale=-0.5)
            nmr4 = small.tile([P, CT, 1], f32, tag="nmr4")
            nc.vector.tensor_mul(out=nmr4, in0=mvs[:, :, 0:1], in1=rstd4)
            nc.vector.tensor_scalar_mul(out=nmr4, in0=nmr4, scalar1=-1.0)
            for mc in range(CT):
                nc.scalar.activation(out=z2[:, mc, :], in_=fwps[mc], func=Ident,
                                     bias=nmr4[:, mc, :], scale=rstd4[:, mc, :])
            st[b]["z2"] = z2

        def emit_t2_final_group(b, kt):
            z2 = st[b]["z2"]
            xb = st[b]["xb"]
            pt2 = ps_t.tile([P, C], mdt, tag="pt")
            for mc in range(CT):
                nc.tensor.transpose(pt2[:, mc * P:(mc + 1) * P],
                                    z2[:, mc, kt * P:(kt + 1) * P], id_sb)
            res = resp.tile([P, C], f32, tag="res")
            nc.scalar.activation(out=res, in_=pt2, func=Ident,
                                 bias=gb_sb[:, kt, 1:2],
                                 scale=gb_sb[:, kt, 0:1])
            nc.vector.tensor_mul(out=res, in0=res, in1=xb[:, kt, :])
            nc.sync.dma_start(out=out_d[b, kt * P:(kt + 1) * P, :], in_=res)
            if kt == KT - 1:
                del st[b]

        # software pipeline, 2-batch skew, with transpose groups woven
        # between independent matmul groups so their psum evictions are
        # hidden behind PE work instead of stalling the pt slots:
        #   cycle b: DCT(b) x T1(b-1) | fc1(b-1) x T2(b-2) | fc2(b-1)
        for b in range(nb + 2):
            if b < nb:
                emit_load(b)
            for g in range(max(CT, KT)):
                if b < nb:
                    emit_dct_ln1_group(b, g)
                if 1 <= b <= nb:
                    emit_t1_group(b - 1, g)
            for mh in range(HT):
                if 1 <= b <= nb:
                    emit_fc1_group(b - 1, mh)
                if b >= 2 and mh % 2 == 1:
                    emit_t2_final_group(b - 2, mh // 2)
            if 1 <= b <= nb:
                emit_fc2_ln2(b - 1)

    # Bacc's compile passes (register alloc, wait splitting for fp32 matmuls)
    # run in finalize(); the pjrt exec path requires a finalized module.
    nc.finalize()
    return nc


def get_nc(nb: int):
    key = (nb, MM_MODE)
    if key not in _NC_CACHE:
        _NC_CACHE[key] = _build(nb)
    return _NC_CACHE[key]


def make_host_inputs(x, gamma, beta, w1, w2):
    """Host-side precompute: DCT matrix + folded weights."""
    x = np.ascontiguousarray(np.asarray(x, dtype=np.float32))
    gamma = np.asarray(gamma, dtype=np.float32)
    beta = np.asarray(beta, dtype=np.float32)
    w1 = np.asarray(w1, dtype=np.float32)
    w2 = np.asarray(w2, dtype=np.float32)

    k = np.arange(C)[:, None].astype(np.float64)
    m = np.arange(C)[None, :].astype(np.float64)
    D = 2.0 * np.cos(np.pi * k * (2.0 * m + 1.0) / (2.0 * C))  # [k, l]
    dt = np.ascontiguousarray(D.T.astype(np.float32))           # [l, k]

    w1t = np.ascontiguousarray((w1 * gamma[None, :]).T)         # [k, h]
    b1 = (w1 @ beta).astype(np.float32)                         # [h]
    w2t = np.ascontiguousarray(w2.T)                            # [h, k]
    gb = np.ascontiguousarray(np.stack([gamma, beta], axis=1))  # [k, 2]
    iden = np.eye(P, dtype=np.float32)
    return x, dict(dt=dt, w1t=w1t, b1=b1, w2t=w2t, gb=gb, iden=iden)


def make_in_maps(x, const):
    nb = B_FULL // N_CORES
    return [dict(x=x[i * nb:(i + 1) * nb], **const) for i in range(N_CORES)]


def kernel(x, gamma, beta, w1, w2):
    from concourse.bass_utils import run_bass_kernel_spmd

    x, const = make_host_inputs(x, gamma, beta, w1, w2)
    nc = get_nc(B_FULL // N_CORES)
    in_maps = make_in_maps(x, const)
    r = run_bass_kernel_spmd(nc, in_maps, list(range(N_CORES)))
    return np.concatenate([r.results[i]["out"] for i in range(N_CORES)], axis=0)


# revision 20
# speedup vs baseline: 1.0673x; 1.0392x over previous
"""FECAM layer Trainium2 kernel.

Reference computation (per batch element b, X = x[b] in R^{512x512}, layout [l, c]):
    xp   = X^T                                  # [c, l]
    freq = xp @ D^T                             # DCT-II along l      [c, k]
    sd   = LN(freq) * gamma + beta              # LayerNorm over k
    h    = relu(sd @ W1^T)                      # [c, 2C]
    fw   = sigmoid(h @ W2^T)                    # [c, k]
    fw   = LN(fw) * gamma + beta
    out  = (xp * fw)^T = X .* fw^T              # [l, c]  (natural layout)

Device strategy (data parallel, 16 batch elements per core x 8 cores):
  - freq[c,k] = matmul(lhsT=x_b tiles [l,c], rhs=D^T tiles [l,k]) -> psum
  - LN1 stats via bn_stats/bn_aggr; rstd = Exp(-0.5*Ln(var+eps)) on ACT
    (keeps every activation in ONE table set - no ACT_TABLE_LOAD thrash);
    z = Identity(freq*rstd - mu*rstd) eviction on ACT
  - LN1 gamma/beta folded into fc1 on host: W1g[h,k]=w1[h,k]*gamma[k],
    b1[h]=sum_k beta[k]*w1[h,k]
  - z transposed 128x128 via PE (f32r, 1.5 cyc/row) into zT [k,c]
  - fc1: hT = relu(W1g @ zT + b1) in [h,c];  fc2: y = hT^T @ W2^T -> [c,k]
  - sigmoid = reciprocal_approx_fast(1 + Exp(-y)): Exp on ACT, +1 and recip on DVE
  - LN2 stats likewise; z2 = Identity eviction; transpose via PE;
    final affine (gamma/beta per-partition) on ACT; multiply by x on DVE
  - emission is software-pipelined with a 2-batch skew so the PE queue always
    has independent matmul work while a batch's LN/sigmoid chains complete:
      cycle b emits: DCT(b) | T2+final(b-2) | T(b-1) fc1(b-1) fc2(b-1)
All matmuls float32r: fp32 operands streamed at 1 cycle/row at free dim 512;
hardware rounds operands tf32-style -> rel err ~4e-4 vs fp64 reference.
"""

import sys

if "/opt/trn_rl_repo" not in sys.path:
    sys.path.insert(0, "/opt/trn_rl_repo")

import numpy as np

P = 128
C = 512          # channels == seq len == dct size
H = 1024         # hidden
CT = C // P      # 4 c-tiles
KT = C // P      # 4 k-tiles
HT = H // P      # 8 h-tiles
EPS = 1e-6
N_CORES = 8
B_FULL = 128

_NC_CACHE: dict = {}

MM_MODE = "f32r"


def _build(nb: int):
    import concourse.bass as bass
    from concourse import bacc
    import concourse.mybir as mybir
    from concourse.tile import TileContext

    f32 = mybir.dt.float32
    f32r = mybir.dt.float32r
    Relu = mybir.ActivationFunctionType.Relu
    Ln = mybir.ActivationFunctionType.Ln
    Exp = mybir.ActivationFunctionType.Exp
    Ident = mybir.ActivationFunctionType.Identity
    mult = mybir.AluOpType.mult
    add = mybir.AluOpType.add

    mdt = f32r if MM_MODE == "f32r" else f32

    nc = bacc.Bacc()
    x_d = nc.declare_dram_parameter("x", [nb, C, C], mdt, isOutput=False)
    dt_d = nc.declare_dram_parameter("dt", [C, C], mdt, isOutput=False)
    w1t_d = nc.declare_dram_parameter("w1t", [C, H], mdt, isOutput=False)
    b1_d = nc.declare_dram_parameter("b1", [H], f32, isOutput=False)
    w2t_d = nc.declare_dram_parameter("w2t", [H, C], mdt, isOutput=False)
    gb_d = nc.declare_dram_parameter("gb", [C, 2], f32, isOutput=False)
    id_d = nc.declare_dram_parameter("iden", [P, P], mdt, isOutput=False)
    out_d = nc.declare_dram_parameter("out", [nb, C, C], f32, isOutput=True)

    with TileContext(nc) as tc, \
            tc.tile_pool(name="consts", bufs=1) as consts, \
            tc.tile_pool(name="xin", bufs=4) as xin, \
            tc.tile_pool(name="work", bufs=2) as work, \
            tc.tile_pool(name="small", bufs=8) as small, \
            tc.tile_pool(name="res", bufs=4) as resp, \
            tc.tile_pool(name="ps_mm", bufs=2, space="PSUM") as ps_mm, \
            tc.tile_pool(name="ps_t", bufs=2, space="PSUM") as ps_t, \
            tc.tile_pool(name="ps_h", bufs=2, space="PSUM") as ps_h, \
            tc.tile_pool(name="ps_w", bufs=2, space="PSUM") as ps_w:

        # one ACT table set covering Ln/Exp/Identity/Copy/Relu: pre-seed it so
        # bacc's availability pass never inserts another load
        from concourse.hw_specs import get_activation_tables
        set_names = list(get_activation_tables(nc.m.arch))
        nc.scalar.add_instruction(mybir.InstLoadActFuncSet(
            name=nc.get_next_instruction_name(),
            act_func_set_id=set_names.index("natural_log_exp_and_others"),
            ins=[], outs=[]))

        dt_sb = consts.tile([P, KT, C], mdt)
        nc.sync.dma_start(out=dt_sb, in_=dt_d.rearrange("(t p) k -> p t k", p=P))
        w1t_sb = consts.tile([P, KT, H], mdt)
        nc.sync.dma_start(out=w1t_sb, in_=w1t_d.rearrange("(t p) h -> p t h", p=P))
        w2t_sb = consts.tile([P, HT, C], mdt)
        nc.sync.dma_start(out=w2t_sb, in_=w2t_d.rearrange("(t p) k -> p t k", p=P))
        b1_sb = consts.tile([P, HT], f32)
        nc.sync.dma_start(out=b1_sb, in_=b1_d.rearrange("(t p) -> p t", p=P))
        gb_sb = consts.tile([P, KT, 2], f32)
        nc.sync.dma_start(out=gb_sb, in_=gb_d.rearrange("(t p) g -> p t g", p=P))
        id_sb = consts.tile([P, P], mdt)
        nc.sync.dma_start(out=id_sb, in_=id_d[:])
        eps_sb = consts.tile([P, 1], f32)
        nc.vector.memset(eps_sb, EPS)

        st: dict = {}   # per-batch live tiles

        def ln_rstd_nmr(mv):
            """(rstd, -mu*rstd) from bn_aggr output, Ln/Exp on ACT."""
            lv = small.tile([P, 1], f32, tag="lv")
            nc.scalar.activation(out=lv, in_=mv[:, 1:2], func=Ln,
                                 bias=eps_sb, scale=1.0)
            rstd = small.tile([P, 1], f32, tag="rstd")
            nc.scalar.activation(out=rstd, in_=lv, func=Exp,
                                 bias=0.0, scale=-0.5)
            nmr = small.tile([P, 1], f32, tag="nmr")
            nc.vector.tensor_scalar(out=nmr, in0=mv[:, 0:1],
                                    scalar1=rstd, scalar2=-1.0,
                                    op0=mult, op1=mult)
            return rstd, nmr

        def emit_load(b):
            xb = xin.tile([P, KT, C], mdt, tag="xb")
            nc.sync.dma_start(out=xb, in_=x_d[b].rearrange("(t p) c -> p t c", p=P))
            st[b] = {"xb": xb}

        def emit_dct_ln1_group(b, mc):
            if mc == 0:
                z_new = work.tile([P, CT, C], mdt, tag="z")
                st[b]["z"] = z_new
            xb = st[b]["xb"]
            z = st[b]["z"]
            pf = ps_mm.tile([P, C], f32, tag="pf")
            for lt in range(KT):
                nc.tensor.matmul(
                    pf,
                    lhsT=xb[:, lt, mc * P:(mc + 1) * P],
                    rhs=dt_sb[:, lt, :],
                    start=(lt == 0),
                    stop=(lt == KT - 1),
                )
            stats = small.tile([P, 6], f32, tag="stats")
            nc.vector.bn_stats(out=stats, in_=pf)
            mv = small.tile([P, 2], f32, tag="mv")
            nc.vector.bn_aggr(out=mv, in_=stats)
            rstd, nmr = ln_rstd_nmr(mv)
            nc.scalar.activation(out=z[:, mc, :], in_=pf, func=Ident,
                                 bias=nmr, scale=rstd)

        def emit_t1_group(b, kt):
            if kt == 0:
                zT_new = work.tile([P, KT, C], mdt, tag="zT")
                st[b]["zT"] = zT_new
            z = st[b]["z"]
            zT = st[b]["zT"]
            pt = ps_t.tile([P, C], mdt, tag="pt")
            for mc in range(CT):
                nc.tensor.transpose(pt[:, mc * P:(mc + 1) * P],
                                    z[:, mc, kt * P:(kt + 1) * P], id_sb)
            nc.scalar.copy(out=zT[:, kt, :], in_=pt)
            if kt == KT - 1:
                del st[b]["z"]

        def emit_fc1_group(b, mh):
            if mh == 0:
                hT_new = work.tile([P, HT, C], mdt, tag="hT")
                st[b]["hT"] = hT_new
            zT = st[b]["zT"]
            hT = st[b]["hT"]
            ph = ps_h.tile([P, C], f32, tag="ph")
            for kt in range(KT):
                nc.tensor.matmul(
                    ph,
                    lhsT=w1t_sb[:, kt, mh * P:(mh + 1) * P],
                    rhs=zT[:, kt, :],
                    start=(kt == 0),
                    stop=(kt == KT - 1),
                )
            nc.scalar.activation(out=hT[:, mh, :], in_=ph, func=Relu,
                                 bias=b1_sb[:, mh:mh + 1], scale=1.0)
            if mh == HT - 1:
                del st[b]["zT"]

        def emit_fc2_ln2(b):
            hT = st[b].pop("hT")
            z2 = work.tile([P, CT, C], mdt, tag="z2")
            for mc in range(CT):
                pw = ps_w.tile([P, C], f32, tag="pw")
                for ht in range(HT):
                    nc.tensor.matmul(
                        pw,
                        lhsT=hT[:, ht, mc * P:(mc + 1) * P],
                        rhs=w2t_sb[:, ht, :],
                        start=(ht == 0),
                        stop=(ht == HT - 1),
                    )
                et = work.tile([P, C], f32, tag="et")
                nc.scalar.activation(out=et, in_=pw, func=Exp,
                                     bias=0.0, scale=-1.0)
                nc.vector.tensor_scalar_add(out=et, in0=et, scalar1=1.0)
                fwp = work.tile([P, C], f32, tag="fwp")
                nc.vector.reciprocal_approx_fast(out=fwp, in_=et)
                stats2 = small.tile([P, 6], f32, tag="stats")
                nc.vector.bn_stats(out=stats2, in_=fwp)
                mv2 = small.tile([P, 2], f32, tag="mv")
                nc.vector.bn_aggr(out=mv2, in_=stats2)
                rstd2, nmr2 = ln_rstd_nmr(mv2)
                nc.scalar.activation(out=z2[:, mc, :], in_=fwp, func=Ident,
                                     bias=nmr2, scale=rstd2)
            st[b]["z2"] = z2

        def emit_t2_final_group(b, kt):
            z2 = st[b]["z2"]
            xb = st[b]["xb"]
            pt2 = ps_t.tile([P, C], mdt, tag="pt")
            for mc in range(CT):
                nc.tensor.transpose(pt2[:, mc * P:(mc + 1) * P],
                                    z2[:, mc, kt * P:(kt + 1) * P], id_sb)
            res = resp.tile([P, C], f32, tag="res")
            nc.scalar.activation(out=res, in_=pt2, func=Ident,
                                 bias=gb_sb[:, kt, 1:2],
                                 scale=gb_sb[:, kt, 0:1])
            nc.vector.tensor_mul(out=res, in0=res, in1=xb[:, kt, :])
            nc.sync.dma_start(out=out_d[b, kt * P:(kt + 1) * P, :], in_=res)
            if kt == KT - 1:
                del st[b]

        # software pipeline, 2-batch skew, with transpose groups woven
        # between independent matmul groups so their psum evictions are
        # hidden behind PE work instead of stalling the pt slots:
        #   cycle b: DCT(b) x T1(b-1) | fc1(b-1) x T2(b-2) | fc2(b-1)
        for b in range(nb + 2):
            if b < nb:
                emit_load(b)
            for g in range(max(CT, KT)):
                if b < nb:
                    emit_dct_ln1_group(b, g)
                if 1 <= b <= nb:
                    emit_t1_group(b - 1, g)
            for mh in range(HT):
                if 1 <= b <= nb:
                    emit_fc1_group(b - 1, mh)
                if b >= 2 and mh % 2 == 1:
                    emit_t2_final_group(b - 2, mh // 2)
            if 1 <= b <= nb:
                emit_fc2_ln2(b - 1)

    # Bacc's compile passes (register alloc, wait splitting for fp32 matmuls)
    # run in finalize(); the pjrt exec path requires a finalized module.
    nc.finalize()
    return nc


def get_nc(nb: int):
    key = (nb, MM_MODE)
    if key not in _NC_CACHE:
        _NC_CACHE[key] = _build(nb)
    return _NC_CACHE[key]


def make_host_inputs(x, gamma, beta, w1, w2):
    """Host-side precompute: DCT matrix + folded weights."""
    x = np.ascontiguousarray(np.asarray(x, dtype=np.float32))
    gamma = np.asarray(gamma, dtype=np.float32)
    beta = np.asarray(beta, dtype=np.float32)
    w1 = np.asarray(w1, dtype=np.float32)
    w2 = np.asarray(w2, dtype=np.float32)

    k = np.arange(C)[:, None].astype(np.float64)
    m = np.arange(C)[None, :].astype(np.float64)
    D = 2.0 * np.cos(np.pi * k * (2.0 * m + 1.0) / (2.0 * C))  # [k, l]
    dt = np.ascontiguousarray(D.T.astype(np.float32))           # [l, k]

    w1t = np.ascontiguousarray((w1 * gamma[None, :]).T)         # [k, h]
    b1 = (w1 @ beta).astype(np.float32)                         # [h]
    w2t = np.ascontiguousarray(w2.T)                            # [h, k]
    gb = np.ascontiguousarray(np.stack([gamma, beta], axis=1))  # [k, 2]
    iden = np.eye(P, dtype=np.float32)
    return x, dict(dt=dt, w1t=w1t, b1=b1, w2t=w2t, gb=gb, iden=iden)


def make_in_maps(x, const):
    nb = B_FULL // N_CORES
    return [dict(x=x[i * nb:(i + 1) * nb], **const) for i in range(N_CORES)]


def kernel(x, gamma, beta, w1, w2):
    from concourse.bass_utils import run_bass_kernel_spmd

    x, const = make_host_inputs(x, gamma, beta, w1, w2)
    nc = get_nc(B_FULL // N_CORES)
    in_maps = make_in_maps(x, const)
    r = run_bass_kernel_spmd(nc, in_maps, list(range(N_CORES)))
    return np.concatenate([r.results[i]["out"] for i in range(N_CORES)], axis=0)


# revision 21
# speedup vs baseline: 1.0753x; 1.0076x over previous
"""FECAM layer Trainium2 kernel.

Reference computation (per batch element b, X = x[b] in R^{512x512}, layout [l, c]):
    xp   = X^T                                  # [c, l]
    freq = xp @ D^T                             # DCT-II along l      [c, k]
    sd   = LN(freq) * gamma + beta              # LayerNorm over k
    h    = relu(sd @ W1^T)                      # [c, 2C]
    fw   = sigmoid(h @ W2^T)                    # [c, k]
    fw   = LN(fw) * gamma + beta
    out  = (xp * fw)^T = X .* fw^T              # [l, c]  (natural layout)

Device strategy (data parallel, 16 batch elements per core x 8 cores):
  - freq[c,k] = matmul(lhsT=x_b tiles [l,c], rhs=D^T tiles [l,k]) -> psum
  - LN1 stats via bn_stats/bn_aggr; rstd = Exp(-0.5*Ln(var+eps)) on ACT
    (keeps every activation in ONE table set - no ACT_TABLE_LOAD thrash);
    z = Identity(freq*rstd - mu*rstd) eviction on ACT
  - LN1 gamma/beta folded into fc1 on host: W1g[h,k]=w1[h,k]*gamma[k],
    b1[h]=sum_k beta[k]*w1[h,k]
  - z transposed 128x128 via PE (f32r, 1.5 cyc/row) into zT [k,c]
  - fc1: hT = relu(W1g @ zT + b1) in [h,c];  fc2: y = hT^T @ W2^T -> [c,k]
  - sigmoid = reciprocal_approx_fast(1 + Exp(-y)): Exp on ACT, +1 and recip on DVE
  - LN2 stats likewise; z2 = Identity eviction; transpose via PE;
    final affine (gamma/beta per-partition) on ACT; multiply by x on DVE
  - emission is software-pipelined with a 2-batch skew so the PE queue always
    has independent matmul work while a batch's LN/sigmoid chains complete:
      cycle b emits: DCT(b) | T2+final(b-2) | T(b-1) fc1(b-1) fc2(b-1)
All matmuls float32r: fp32 operands streamed at 1 cycle/row at free dim 512;
hardware rounds operands tf32-style -> rel err ~4e-4 vs fp64 reference.
"""

import sys

if "/opt/trn_rl_repo" not in sys.path:
    sys.path.insert(0, "/opt/trn_rl_repo")

import numpy as np

P = 128
C = 512          # channels == seq len == dct size
H = 1024         # hidden
CT = C // P      # 4 c-tiles
KT = C // P      # 4 k-tiles
HT = H // P      # 8 h-tiles
EPS = 1e-6
N_CORES = 8
B_FULL = 128

_NC_CACHE: dict = {}

MM_MODE = "f32r"


def _build(nb: int):
    import concourse.bass as bass
    from concourse import bacc
    import concourse.mybir as mybir
    from concourse.tile import TileContext

    f32 = mybir.dt.float32
    f32r = mybir.dt.float32r
    Relu = mybir.ActivationFunctionType.Relu
    Ln = mybir.ActivationFunctionType.Ln
    Exp = mybir.ActivationFunctionType.Exp
    Ident = mybir.ActivationFunctionType.Identity
    mult = mybir.AluOpType.mult
    add = mybir.AluOpType.add
    sub = mybir.AluOpType.subtract

    mdt = f32r if MM_MODE == "f32r" else f32

    nc = bacc.Bacc()
    x_d = nc.declare_dram_parameter("x", [nb, C, C], mdt, isOutput=False)
    dt_d = nc.declare_dram_parameter("dt", [C, C], mdt, isOutput=False)
    w1t_d = nc.declare_dram_parameter("w1t", [C, H], mdt, isOutput=False)
    b1_d = nc.declare_dram_parameter("b1", [H], f32, isOutput=False)
    w2t_d = nc.declare_dram_parameter("w2t", [H, C], mdt, isOutput=False)
    gb_d = nc.declare_dram_parameter("gb", [C, 2], f32, isOutput=False)
    id_d = nc.declare_dram_parameter("iden", [P, P], mdt, isOutput=False)
    out_d = nc.declare_dram_parameter("out", [nb, C, C], f32, isOutput=True)

    with TileContext(nc) as tc, \
            tc.tile_pool(name="consts", bufs=1) as consts, \
            tc.tile_pool(name="xin", bufs=4) as xin, \
            tc.tile_pool(name="work", bufs=2) as work, \
            tc.tile_pool(name="small", bufs=8) as small, \
            tc.tile_pool(name="res", bufs=4) as resp, \
            tc.tile_pool(name="ps_mm", bufs=2, space="PSUM") as ps_mm, \
            tc.tile_pool(name="ps_t", bufs=2, space="PSUM") as ps_t, \
            tc.tile_pool(name="ps_h", bufs=2, space="PSUM") as ps_h, \
            tc.tile_pool(name="ps_w", bufs=2, space="PSUM") as ps_w:

        # one ACT table set covering Ln/Exp/Identity/Copy/Relu: pre-seed it so
        # bacc's availability pass never inserts another load
        from concourse.hw_specs import get_activation_tables
        set_names = list(get_activation_tables(nc.m.arch))
        nc.scalar.add_instruction(mybir.InstLoadActFuncSet(
            name=nc.get_next_instruction_name(),
            act_func_set_id=set_names.index("natural_log_exp_and_others"),
            ins=[], outs=[]))

        dt_sb = consts.tile([P, KT, C], mdt)
        nc.sync.dma_start(out=dt_sb, in_=dt_d.rearrange("(t p) k -> p t k", p=P))
        w1t_sb = consts.tile([P, KT, H], mdt)
        nc.sync.dma_start(out=w1t_sb, in_=w1t_d.rearrange("(t p) h -> p t h", p=P))
        w2t_sb = consts.tile([P, HT, C], mdt)
        nc.sync.dma_start(out=w2t_sb, in_=w2t_d.rearrange("(t p) k -> p t k", p=P))
        b1_sb = consts.tile([P, HT], f32)
        nc.sync.dma_start(out=b1_sb, in_=b1_d.rearrange("(t p) -> p t", p=P))
        gb_sb = consts.tile([P, KT, 2], f32)
        nc.sync.dma_start(out=gb_sb, in_=gb_d.rearrange("(t p) g -> p t g", p=P))
        id_sb = consts.tile([P, P], mdt)
        nc.sync.dma_start(out=id_sb, in_=id_d[:])
        eps_sb = consts.tile([P, 1], f32)
        nc.vector.memset(eps_sb, EPS)

        st: dict = {}   # per-batch live tiles

        def ln_rstd_nmr(mv):
            """(rstd, -mu*rstd) from bn_aggr output, Ln/Exp on ACT."""
            lv = small.tile([P, 1], f32, tag="lv")
            nc.scalar.activation(out=lv, in_=mv[:, 1:2], func=Ln,
                                 bias=eps_sb, scale=1.0)
            rstd = small.tile([P, 1], f32, tag="rstd")
            nc.scalar.activation(out=rstd, in_=lv, func=Exp,
                                 bias=0.0, scale=-0.5)
            nmr = small.tile([P, 1], f32, tag="nmr")
            nc.vector.tensor_scalar(out=nmr, in0=mv[:, 0:1],
                                    scalar1=rstd, scalar2=-1.0,
                                    op0=mult, op1=mult)
            return rstd, nmr

        def emit_load(b):
            xb = xin.tile([P, KT, C], mdt, tag="xb")
            nc.sync.dma_start(out=xb, in_=x_d[b].rearrange("(t p) c -> p t c", p=P))
            st[b] = {"xb": xb}

        def emit_dct_ln1_group(b, mc):
            if mc == 0:
                z_new = work.tile([P, CT, C], mdt, tag="z")
                st[b]["z"] = z_new
            xb = st[b]["xb"]
            z = st[b]["z"]
            pf = ps_mm.tile([P, C], f32, tag="pf")
            for lt in range(KT):
                nc.tensor.matmul(
                    pf,
                    lhsT=xb[:, lt, mc * P:(mc + 1) * P],
                    rhs=dt_sb[:, lt, :],
                    start=(lt == 0),
                    stop=(lt == KT - 1),
                )
            stats = small.tile([P, 6], f32, tag="stats")
            nc.vector.bn_stats(out=stats, in_=pf)
            mv = small.tile([P, 2], f32, tag="mv")
            nc.vector.bn_aggr(out=mv, in_=stats)
            lv = small.tile([P, 1], f32, tag="lv")
            nc.scalar.activation(out=lv, in_=mv[:, 1:2], func=Ln,
                                 bias=eps_sb, scale=1.0)
            rstd = small.tile([P, 1], f32, tag="rstd")
            nc.scalar.activation(out=rstd, in_=lv, func=Exp,
                                 bias=0.0, scale=-0.5)
            nc.vector.tensor_scalar(out=z[:, mc, :], in0=pf,
                                    scalar1=mv[:, 0:1], scalar2=rstd,
                                    op0=sub, op1=mult)

        def emit_t1_group(b, kt):
            if kt == 0:
                zT_new = work.tile([P, KT, C], mdt, tag="zT")
                st[b]["zT"] = zT_new
            z = st[b]["z"]
            zT = st[b]["zT"]
            pt = ps_t.tile([P, C], mdt, tag="pt")
            for mc in range(CT):
                nc.tensor.transpose(pt[:, mc * P:(mc + 1) * P],
                                    z[:, mc, kt * P:(kt + 1) * P], id_sb)
            nc.scalar.copy(out=zT[:, kt, :], in_=pt)
            if kt == KT - 1:
                del st[b]["z"]

        def emit_fc1_group(b, mh):
            if mh == 0:
                hT_new = work.tile([P, HT, C], mdt, tag="hT")
                st[b]["hT"] = hT_new
            zT = st[b]["zT"]
            hT = st[b]["hT"]
            ph = ps_h.tile([P, C], f32, tag="ph")
            for kt in range(KT):
                nc.tensor.matmul(
                    ph,
                    lhsT=w1t_sb[:, kt, mh * P:(mh + 1) * P],
                    rhs=zT[:, kt, :],
                    start=(kt == 0),
                    stop=(kt == KT - 1),
                )
            nc.scalar.activation(out=hT[:, mh, :], in_=ph, func=Relu,
                                 bias=b1_sb[:, mh:mh + 1], scale=1.0)
            if mh == HT - 1:
                del st[b]["zT"]

        def emit_fc2_ln2(b):
            hT = st[b].pop("hT")
            z2 = work.tile([P, CT, C], mdt, tag="z2")
            for mc in range(CT):
                pw = ps_w.tile([P, C], f32, tag="pw")
                for ht in range(HT):
                    nc.tensor.matmul(
                        pw,
                        lhsT=hT[:, ht, mc * P:(mc + 1) * P],
                        rhs=w2t_sb[:, ht, :],
                        start=(ht == 0),
                        stop=(ht == HT - 1),
                    )
                et = work.tile([P, C], f32, tag="et")
                nc.scalar.activation(out=et, in_=pw, func=Exp,
                                     bias=0.0, scale=-1.0)
                nc.vector.tensor_scalar_add(out=et, in0=et, scalar1=1.0)
                fwp = work.tile([P, C], f32, tag="fwp")
                nc.vector.reciprocal_approx_fast(out=fwp, in_=et)
                stats2 = small.tile([P, 6], f32, tag="stats")
                nc.vector.bn_stats(out=stats2, in_=fwp)
                mv2 = small.tile([P, 2], f32, tag="mv")
                nc.vector.bn_aggr(out=mv2, in_=stats2)
                rstd2, nmr2 = ln_rstd_nmr(mv2)
                nc.scalar.activation(out=z2[:, mc, :], in_=fwp, func=Ident,
                                     bias=nmr2, scale=rstd2)
            st[b]["z2"] = z2

        def emit_t2_final_group(b, kt):
            z2 = st[b]["z2"]
            xb = st[b]["xb"]
            pt2 = ps_t.tile([P, C], mdt, tag="pt")
            for mc in range(CT):
                nc.tensor.transpose(pt2[:, mc * P:(mc + 1) * P],
                                    z2[:, mc, kt * P:(kt + 1) * P], id_sb)
            res = resp.tile([P, C], f32, tag="res")
            nc.scalar.activation(out=res, in_=pt2, func=Ident,
                                 bias=gb_sb[:, kt, 1:2],
                                 scale=gb_sb[:, kt, 0:1])
            nc.vector.tensor_mul(out=res, in0=res, in1=xb[:, kt, :])
            nc.sync.dma_start(out=out_d[b, kt * P:(kt + 1) * P, :], in_=res)
            if kt == KT - 1:
                del st[b]

        # software pipeline, 2-batch skew, with transpose groups woven
        # between independent matmul groups so their psum evictions are
        # hidden behind PE work instead of stalling the pt slots:
        #   cycle b: DCT(b) x T1(b-1) | fc1(b-1) x T2(b-2) | fc2(b-1)
        for b in range(nb + 2):
            if b < nb:
                emit_load(b)
            for g in range(max(CT, KT)):
                if b < nb:
                    emit_dct_ln1_group(b, g)
                if 1 <= b <= nb:
                    emit_t1_group(b - 1, g)
            for mh in range(HT):
                if 1 <= b <= nb:
                    emit_fc1_group(b - 1, mh)
                if b >= 2 and mh % 2 == 1:
                    emit_t2_final_group(b - 2, mh // 2)
            if 1 <= b <= nb:
                emit_fc2_ln2(b - 1)

    # Bacc's compile passes (register alloc, wait splitting for fp32 matmuls)
    # run in finalize(); the pjrt exec path requires a finalized module.
    nc.finalize()
    return nc


def get_nc(nb: int):
    key = (nb, MM_MODE)
    if key not in _NC_CACHE:
        _NC_CACHE[key] = _build(nb)
    return _NC_CACHE[key]


def make_host_inputs(x, gamma, beta, w1, w2):
    """Host-side precompute: DCT matrix + folded weights."""
    x = np.ascontiguousarray(np.asarray(x, dtype=np.float32))
    gamma = np.asarray(gamma, dtype=np.float32)
    beta = np.asarray(beta, dtype=np.float32)
    w1 = np.asarray(w1, dtype=np.float32)
    w2 = np.asarray(w2, dtype=np.float32)

    k = np.arange(C)[:, None].astype(np.float64)
    m = np.arange(C)[None, :].astype(np.float64)
    D = 2.0 * np.cos(np.pi * k * (2.0 * m + 1.0) / (2.0 * C))  # [k, l]
    dt = np.ascontiguousarray(D.T.astype(np.float32))           # [l, k]

    w1t = np.ascontiguousarray((w1 * gamma[None, :]).T)         # [k, h]
    b1 = (w1 @ beta).astype(np.float32)                         # [h]
    w2t = np.ascontiguousarray(w2.T)                            # [h, k]
    gb = np.ascontiguousarray(np.stack([gamma, beta], axis=1))  # [k, 2]
    iden = np.eye(P, dtype=np.float32)
    return x, dict(dt=dt, w1t=w1t, b1=b1, w2t=w2t, gb=gb, iden=iden)


def make_in_maps(x, const):
    nb = B_FULL // N_CORES
    return [dict(x=x[i * nb:(i + 1) * nb], **const) for i in range(N_CORES)]


def kernel(x, gamma, beta, w1, w2):
    from concourse.bass_utils import run_bass_kernel_spmd

    x, const = make_host_inputs(x, gamma, beta, w1, w2)
    nc = get_nc(B_FULL // N_CORES)
    in_maps = make_in_maps(x, const)
    r = run_bass_kernel_spmd(nc, in_maps, list(range(N_CORES)))
    return np.concatenate([r.results[i]["out"] for i in range(N_CORES)], axis=0)


# revision 23
# speedup vs baseline: 1.0828x; 1.0069x over previous
"""FECAM layer Trainium2 kernel.

Reference computation (per batch element b, X = x[b] in R^{512x512}, layout [l, c]):
    xp   = X^T                                  # [c, l]
    freq = xp @ D^T                             # DCT-II along l      [c, k]
    sd   = LN(freq) * gamma + beta              # LayerNorm over k
    h    = relu(sd @ W1^T)                      # [c, 2C]
    fw   = sigmoid(h @ W2^T)                    # [c, k]
    fw   = LN(fw) * gamma + beta
    out  = (xp * fw)^T = X .* fw^T              # [l, c]  (natural layout)

Device strategy (data parallel, 16 batch elements per core x 8 cores):
  - freq[c,k] = matmul(lhsT=x_b tiles [l,c], rhs=D^T tiles [l,k]) -> psum
  - LN1 stats via bn_stats/bn_aggr; rstd = Exp(-0.5*Ln(var+eps)) on ACT
    (keeps every activation in ONE table set - no ACT_TABLE_LOAD thrash);
    z = Identity(freq*rstd - mu*rstd) eviction on ACT
  - LN1 gamma/beta folded into fc1 on host: W1g[h,k]=w1[h,k]*gamma[k],
    b1[h]=sum_k beta[k]*w1[h,k]
  - z transposed 128x128 via PE (f32r, 1.5 cyc/row) into zT [k,c]
  - fc1: hT = relu(W1g @ zT + b1) in [h,c];  fc2: y = hT^T @ W2^T -> [c,k]
  - sigmoid = reciprocal_approx_fast(1 + Exp(-y)): Exp on ACT, +1 and recip on DVE
  - LN2 stats likewise; z2 = Identity eviction; transpose via PE;
    final affine (gamma/beta per-partition) on ACT; multiply by x on DVE
  - emission is software-pipelined with a 2-batch skew so the PE queue always
    has independent matmul work while a batch's LN/sigmoid chains complete:
      cycle b emits: DCT(b) | T2+final(b-2) | T(b-1) fc1(b-1) fc2(b-1)
All matmuls float32r: fp32 operands streamed at 1 cycle/row at free dim 512;
hardware rounds operands tf32-style -> rel err ~4e-4 vs fp64 reference.
"""

import sys

if "/opt/trn_rl_repo" not in sys.path:
    sys.path.insert(0, "/opt/trn_rl_repo")

import numpy as np

P = 128
C = 512          # channels == seq len == dct size
H = 1024         # hidden
CT = C // P      # 4 c-tiles
KT = C // P      # 4 k-tiles
HT = H // P      # 8 h-tiles
EPS = 1e-6
N_CORES = 8
B_FULL = 128

_NC_CACHE: dict = {}

MM_MODE = "f32r"


def _build(nb: int):
    import concourse.bass as bass
    from concourse import bacc
    import concourse.mybir as mybir
    from concourse.tile import TileContext

    f32 = mybir.dt.float32
    f32r = mybir.dt.float32r
    Relu = mybir.ActivationFunctionType.Relu
    Ln = mybir.ActivationFunctionType.Ln
    Exp = mybir.ActivationFunctionType.Exp
    Ident = mybir.ActivationFunctionType.Identity
    mult = mybir.AluOpType.mult
    add = mybir.AluOpType.add
    sub = mybir.AluOpType.subtract

    mdt = f32r if MM_MODE == "f32r" else f32

    nc = bacc.Bacc()
    x_d = nc.declare_dram_parameter("x", [nb, C, C], mdt, isOutput=False)
    dt_d = nc.declare_dram_parameter("dt", [C, C], mdt, isOutput=False)
    w1t_d = nc.declare_dram_parameter("w1t", [C, H], mdt, isOutput=False)
    b1_d = nc.declare_dram_parameter("b1", [H], f32, isOutput=False)
    w2t_d = nc.declare_dram_parameter("w2t", [H, C], mdt, isOutput=False)
    gb_d = nc.declare_dram_parameter("gb", [C, 2], f32, isOutput=False)
    id_d = nc.declare_dram_parameter("iden", [P, P], mdt, isOutput=False)
    out_d = nc.declare_dram_parameter("out", [nb, C, C], f32, isOutput=True)

    with TileContext(nc) as tc, \
            tc.tile_pool(name="consts", bufs=1) as consts, \
            tc.tile_pool(name="xin", bufs=4) as xin, \
            tc.tile_pool(name="work", bufs=2) as work, \
            tc.tile_pool(name="small", bufs=8) as small, \
            tc.tile_pool(name="res", bufs=4) as resp, \
            tc.tile_pool(name="ps_mm", bufs=2, space="PSUM") as ps_mm, \
            tc.tile_pool(name="ps_t", bufs=2, space="PSUM") as ps_t, \
            tc.tile_pool(name="ps_h", bufs=2, space="PSUM") as ps_h, \
            tc.tile_pool(name="ps_w", bufs=2, space="PSUM") as ps_w:

        # one ACT table set covering Ln/Exp/Identity/Copy/Relu: pre-seed it so
        # bacc's availability pass never inserts another load
        from concourse.hw_specs import get_activation_tables
        set_names = list(get_activation_tables(nc.m.arch))
        nc.scalar.add_instruction(mybir.InstLoadActFuncSet(
            name=nc.get_next_instruction_name(),
            act_func_set_id=set_names.index("natural_log_exp_and_others"),
            ins=[], outs=[]))

        dt_sb = consts.tile([P, KT, C], mdt)
        nc.sync.dma_start(out=dt_sb, in_=dt_d.rearrange("(t p) k -> p t k", p=P))
        w1t_sb = consts.tile([P, KT, H], mdt)
        nc.sync.dma_start(out=w1t_sb, in_=w1t_d.rearrange("(t p) h -> p t h", p=P))
        w2t_sb = consts.tile([P, HT, C], mdt)
        nc.sync.dma_start(out=w2t_sb, in_=w2t_d.rearrange("(t p) k -> p t k", p=P))
        b1_sb = consts.tile([P, HT], f32)
        nc.sync.dma_start(out=b1_sb, in_=b1_d.rearrange("(t p) -> p t", p=P))
        gb_sb = consts.tile([P, KT, 2], f32)
        nc.sync.dma_start(out=gb_sb, in_=gb_d.rearrange("(t p) g -> p t g", p=P))
        id_sb = consts.tile([P, P], mdt)
        nc.sync.dma_start(out=id_sb, in_=id_d[:])
        eps_sb = consts.tile([P, 1], f32)
        nc.vector.memset(eps_sb, EPS)

        st: dict = {}   # per-batch live tiles

        def ln_rstd_nmr(mv):
            """(rstd, -mu*rstd) from bn_aggr output, Ln/Exp on ACT."""
            lv = small.tile([P, 1], f32, tag="lv")
            nc.scalar.activation(out=lv, in_=mv[:, 1:2], func=Ln,
                                 bias=eps_sb, scale=1.0)
            rstd = small.tile([P, 1], f32, tag="rstd")
            nc.scalar.activation(out=rstd, in_=lv, func=Exp,
                                 bias=0.0, scale=-0.5)
            nmr = small.tile([P, 1], f32, tag="nmr")
            nc.vector.tensor_scalar(out=nmr, in0=mv[:, 0:1],
                                    scalar1=rstd, scalar2=-1.0,
                                    op0=mult, op1=mult)
            return rstd, nmr

        def emit_load(b):
            xb = xin.tile([P, KT, C], mdt, tag="xb")
            nc.sync.dma_start(out=xb, in_=x_d[b].rearrange("(t p) c -> p t c", p=P))
            st[b] = {"xb": xb}

        def emit_dct_ln1_group(b, mc):
            if mc == 0:
                z_new = work.tile([P, CT, C], mdt, tag="z")
                st[b]["z"] = z_new
            xb = st[b]["xb"]
            z = st[b]["z"]
            pf = ps_mm.tile([P, C], f32, tag="pf")
            for lt in range(KT):
                nc.tensor.matmul(
                    pf,
                    lhsT=xb[:, lt, mc * P:(mc + 1) * P],
                    rhs=dt_sb[:, lt, :],
                    start=(lt == 0),
                    stop=(lt == KT - 1),
                )
            stats = small.tile([P, 6], f32, tag="stats")
            nc.vector.bn_stats(out=stats, in_=pf)
            mv = small.tile([P, 2], f32, tag="mv")
            nc.vector.bn_aggr(out=mv, in_=stats)
            lv = small.tile([P, 1], f32, tag="lv")
            nc.scalar.activation(out=lv, in_=mv[:, 1:2], func=Ln,
                                 bias=eps_sb, scale=1.0)
            rstd = small.tile([P, 1], f32, tag="rstd")
            nc.scalar.activation(out=rstd, in_=lv, func=Exp,
                                 bias=0.0, scale=-0.5)
            nc.vector.tensor_scalar(out=z[:, mc, :], in0=pf,
                                    scalar1=mv[:, 0:1], scalar2=rstd,
                                    op0=sub, op1=mult)

        def emit_t1_group(b, kt):
            if "zT" not in st[b]:
                zT_new = work.tile([P, KT, C], mdt, tag="zT")
                st[b]["zT"] = zT_new
                st[b]["t1done"] = 0
            z = st[b]["z"]
            zT = st[b]["zT"]
            pt = ps_t.tile([P, C], mdt, tag="pt")
            for mc in range(CT):
                nc.tensor.transpose(pt[:, mc * P:(mc + 1) * P],
                                    z[:, mc, kt * P:(kt + 1) * P], id_sb)
            nc.scalar.copy(out=zT[:, kt, :], in_=pt)
            st[b]["t1done"] += 1
            if st[b]["t1done"] == KT:
                del st[b]["z"]
                del st[b]["t1done"]

        def emit_fc1_group(b, mh):
            if mh == 0:
                hT_new = work.tile([P, HT, C], mdt, tag="hT")
                st[b]["hT"] = hT_new
            zT = st[b]["zT"]
            hT = st[b]["hT"]
            ph = ps_h.tile([P, C], f32, tag="ph")
            for kt in range(KT):
                nc.tensor.matmul(
                    ph,
                    lhsT=w1t_sb[:, kt, mh * P:(mh + 1) * P],
                    rhs=zT[:, kt, :],
                    start=(kt == 0),
                    stop=(kt == KT - 1),
                )
            nc.scalar.activation(out=hT[:, mh, :], in_=ph, func=Relu,
                                 bias=b1_sb[:, mh:mh + 1], scale=1.0)
            if mh == HT - 1:
                del st[b]["zT"]

        def emit_fc2_ln2(b):
            hT = st[b].pop("hT")
            z2 = work.tile([P, CT, C], mdt, tag="z2")
            for mc in range(CT):
                pw = ps_w.tile([P, C], f32, tag="pw")
                for ht in range(HT):
                    nc.tensor.matmul(
                        pw,
                        lhsT=hT[:, ht, mc * P:(mc + 1) * P],
                        rhs=w2t_sb[:, ht, :],
                        start=(ht == 0),
                        stop=(ht == HT - 1),
                    )
                et = work.tile([P, C], f32, tag="et")
                nc.scalar.activation(out=et, in_=pw, func=Exp,
                                     bias=0.0, scale=-1.0)
                nc.vector.tensor_scalar_add(out=et, in0=et, scalar1=1.0)
                fwp = work.tile([P, C], f32, tag="fwp")
                nc.vector.reciprocal_approx_fast(out=fwp, in_=et)
                stats2 = small.tile([P, 6], f32, tag="stats")
                nc.vector.bn_stats(out=stats2, in_=fwp)
                mv2 = small.tile([P, 2], f32, tag="mv")
                nc.vector.bn_aggr(out=mv2, in_=stats2)
                rstd2, nmr2 = ln_rstd_nmr(mv2)
                nc.scalar.activation(out=z2[:, mc, :], in_=fwp, func=Ident,
                                     bias=nmr2, scale=rstd2)
            st[b]["z2"] = z2

        def emit_t2_final_group(b, kt):
            z2 = st[b]["z2"]
            xb = st[b]["xb"]
            pt2 = ps_t.tile([P, C], mdt, tag="pt")
            for mc in range(CT):
                nc.tensor.transpose(pt2[:, mc * P:(mc + 1) * P],
                                    z2[:, mc, kt * P:(kt + 1) * P], id_sb)
            res = resp.tile([P, C], f32, tag="res")
            nc.scalar.activation(out=res, in_=pt2, func=Ident,
                                 bias=gb_sb[:, kt, 1:2],
                                 scale=gb_sb[:, kt, 0:1])
            nc.vector.tensor_mul(out=res, in0=res, in1=xb[:, kt, :])
            nc.sync.dma_start(out=out_d[b, kt * P:(kt + 1) * P, :], in_=res)
            if kt == KT - 1:
                del st[b]

        # software pipeline, 2-batch skew, with transpose groups woven
        # between independent matmul groups so their psum evictions are
        # hidden behind PE work instead of stalling the pt slots:
        #   cycle b: DCT(b) x T1(b-1) | fc1(b-1) x T2(b-2) | fc2(b-1)
        for b in range(nb + 2):
            if b < nb:
                emit_load(b)
            # T1 emitted BEFORE the paired DCT group, rotated so the last-
            # needed zT chunk (kt=3) is produced first: fc1's first group no
            # longer waits on the last transpose eviction
            kt_rot = [3, 0, 1, 2]
            for g in range(max(CT, KT)):
                if 1 <= b <= nb:
                    emit_t1_group(b - 1, kt_rot[g])
                if b < nb:
                    emit_dct_ln1_group(b, g)
            for mh in range(HT):
                if 1 <= b <= nb:
                    emit_fc1_group(b - 1, mh)
                if b >= 2 and mh % 2 == 1:
                    emit_t2_final_group(b - 2, mh // 2)
            if 1 <= b <= nb:
                emit_fc2_ln2(b - 1)

    # Bacc's compile passes (register alloc, wait splitting for fp32 matmuls)
    # run in finalize(); the pjrt exec path requires a finalized module.
    nc.finalize()
    return nc


def get_nc(nb: int):
    key = (nb, MM_MODE)
    if key not in _NC_CACHE:
        _NC_CACHE[key] = _build(nb)
    return _NC_CACHE[key]


def make_host_inputs(x, gamma, beta, w1, w2):
    """Host-side precompute: DCT matrix + folded weights."""
    x = np.ascontiguousarray(np.asarray(x, dtype=np.float32))
    gamma = np.asarray(gamma, dtype=np.float32)
    beta = np.asarray(beta, dtype=np.float32)
    w1 = np.asarray(w1, dtype=np.float32)
    w2 = np.asarray(w2, dtype=np.float32)

    k = np.arange(C)[:, None].astype(np.float64)
    m = np.arange(C)[None, :].astype(np.float64)
    D = 2.0 * np.cos(np.pi * k * (2.0 * m + 1.0) / (2.0 * C))  # [k, l]
    dt = np.ascontiguousarray(D.T.astype(np.float32))           # [l, k]

    w1t = np.ascontiguousarray((w1 * gamma[None, :]).T)         # [k, h]
    b1 = (w1 @ beta).astype(np.float32)                         # [h]
    w2t = np.ascontiguousarray(w2.T)                            # [h, k]
    gb = np.ascontiguousarray(np.stack([gamma, beta], axis=1))  # [k, 2]
    iden = np.eye(P, dtype=np.float32)
    return x, dict(dt=dt, w1t=w1t, b1=b1, w2t=w2t, gb=gb, iden=iden)


def make_in_maps(x, const):
    nb = B_FULL // N_CORES
    return [dict(x=x[i * nb:(i + 1) * nb], **const) for i in range(N_CORES)]


def kernel(x, gamma, beta, w1, w2):
    from concourse.bass_utils import run_bass_kernel_spmd

    x, const = make_host_inputs(x, gamma, beta, w1, w2)
    nc = get_nc(B_FULL // N_CORES)
    in_maps = make_in_maps(x, const)
    r = run_bass_kernel_spmd(nc, in_maps, list(range(N_CORES)))
    return np.concatenate([r.results[i]["out"] for i in range(N_CORES)], axis=0)
